# revision 58
# baseline (speedup 1.0000x reference)
"""SpecAugment (log-mel masking) Trainium2 kernel.

Full inputs: x [64,128,3000] f32, f0/f_w/t0/t_w [64,2] i32.
out[b,f,t] = fill_b if (f in freq band) or (t in time band) else x[b,f,t],
fill_b = min over x[b].

Strategy: batch-shard B=64 across 8 cores (8 samples/core). The int mask
params are tiny host tensors, so the per-sample 0/1 mask vectors are
computed on host and shipped as bf16 data; the device does only the
memory-bound work (2 x 12.3MB HBM traffic per core, ~69us roofline at
358 GB/s).

Per-core schedule (chosen so the serial DVE stream — the critical
path at ~56us busy — starts as early as possible and never stalls):
  - sample 0 arrives as four [128,750] quarter DMAs (sync queue, first)
    so the first DVE reduce starts ~11.5us in; mask rows for all
    samples follow in 2 tiny DMAs, then sample 1 as two halves and
    samples 2..7 as single [128,3000] DMAs (fewer per-op fixed costs)
  - DVE: free-axis min-reduce per chunk -> colmin.  Emission is
    software-pipelined (reduce(b+1) | preds(b)) so predicated copies
    never queue behind later samples' reduces.
  - GpSimd: negate colmin, partition_all_reduce(max) across partitions,
    fold + negate -> fill128 [128,1] on every partition.  A dummy
    partition_all_reduce at kernel start pre-loads the GpSimd ucode
    library (~12us) under the input stream, off the critical path.
  - PE: combined mask = ones(x)mt + mf(x)ones as K=2 bf16 matmuls per
    512-col chunk into PSUM (values {0,1,2}; nonzero == masked)
  - DVE: copy_predicated overwrites masked cells with fill
  - each half stores to y as soon as its pred completes (scalar queue);
    the very last half goes in two quarter chunks to shorten the tail
"""

import ml_dtypes
import numpy as np

import concourse.bacc as bacc
import concourse.bass as bass
import concourse.bass_isa as bass_isa
import concourse.mybir as mybir
import concourse.tile as tile
import concourse.bass_utils as bass_utils

B, F, T = 64, 128, 3000
N_CORES = 8
BPC = B // N_CORES  # samples per core
H = T // 2
F32 = mybir.dt.float32
BF16 = mybir.dt.bfloat16

_cached = {}


def _build_nc():
    nc = bacc.Bacc("TRN2", target_bir_lowering=False, debug=False)
    x = nc.dram_tensor("x_sh", [BPC, F, T], F32, kind="ExternalInput")
    # row 0 = per-sample time masks (0/1) concatenated, row 1 = ones
    mtr = nc.dram_tensor("mtr_sh", [2, BPC * T], BF16, kind="ExternalInput")
    # row 0 = ones, row 1 = per-sample freq masks (0/1) concatenated
    mfl = nc.dram_tensor("mfl_sh", [2, BPC * F], BF16, kind="ExternalInput")
    y = nc.dram_tensor("y_sh", [BPC, F, T], F32, kind="ExternalOutput")

    xa, ta, fa, ya = x.ap(), mtr.ap(), mfl.ap(), y.ap()

    with tile.TileContext(nc) as tc:
        with (
            tc.tile_pool(name="xp", bufs=8) as xp,
            tc.tile_pool(name="small", bufs=8) as sp,
            tc.tile_pool(name="single", bufs=1) as single,
            tc.tile_pool(name="ps", bufs=2, space="PSUM") as psp,
        ):
            # warm up the GpSimd custom-op library: the first custom op
            # triggers a ~12us ucode library load; issue dummy ones
            # immediately so the load overlaps the input stream instead
            # of blocking the first fill/pred
            warm_in = single.tile([F, 1], F32)
            nc.gpsimd.memset(warm_in, 0.0)
            warm_out = single.tile([F, 1], F32)
            nc.gpsimd.partition_all_reduce(
                warm_out, warm_in, channels=F, reduce_op=bass_isa.ReduceOp.max
            )

            xts = [None] * BPC
            cms = [None] * BPC
            fills = [None] * BPC
            def emit_input(b, nchunks=2):
                xt = xp.tile([F, T], F32, tag="xt")
                xts[b] = xt
                w = T // nchunks
                for h in range(nchunks):
                    nc.sync.dma_start(
                        out=xt[:, h * w : (h + 1) * w],
                        in_=xa[b][:, h * w : (h + 1) * w],
                    )

            def emit_reduce(b, h, nchunks=2):
                if cms[b] is None:
                    cm = sp.tile([F, nchunks], F32, tag=f"cm{nchunks}")
                    cms[b] = cm
                w = T // nchunks
                nc.vector.tensor_reduce(
                    out=cms[b][:, h : h + 1],
                    in_=xts[b][:, h * w : (h + 1) * w],
                    axis=mybir.AxisListType.X,
                    op=mybir.AluOpType.min,
                )

            def emit_fill(b):
                nch = cms[b].shape[1]
                cmn = sp.tile([F, nch], F32, tag=f"cmn{nch}")
                nc.gpsimd.tensor_scalar_mul(cmn, cms[b], -1.0)
                nmax = sp.tile([F, nch], F32, tag=f"nmax{nch}")
                nc.gpsimd.partition_all_reduce(
                    nmax, cmn, channels=F, reduce_op=bass_isa.ReduceOp.max
                )
                acc = nmax[:, 0:1]
                for j in range(1, nch - 1):
                    t = sp.tile([F, 1], F32, tag=f"fold{j}")
                    nc.gpsimd.tensor_scalar_max(t, acc, nmax[:, j : j + 1])
                    acc = t
                fill128 = sp.tile([F, 1], F32, tag="fill128")
                # fill = -max(columns), on every partition
                nc.gpsimd.tensor_scalar(
                    out=fill128,
                    in0=acc,
                    scalar1=nmax[:, nch - 1 : nch],
                    scalar2=-1.0,
                    op0=mybir.AluOpType.max,
                    op1=mybir.AluOpType.mult,
                )
                fills[b] = fill128

            def emit_pred_store(b):
                xt = xts[b]
                for h in range(2):
                    msh = psp.tile([F, H], F32, tag="ms")
                    for c0 in range(0, H, 512):
                        cw = min(512, H - c0)
                        nc.tensor.matmul(
                            msh[:, c0 : c0 + cw],
                            mfl_t[:, b * F : (b + 1) * F],
                            mtr_t[:, b * T + h * H + c0 : b * T + h * H + c0 + cw],
                            start=True,
                            stop=True,
                        )
                    # on the very last half, predicate+store unevenly so
                    # the small final chunk's store (and its ~2us receipt)
                    # starts as early as possible (shorter kernel tail)
                    widths = [1125, 375] if (b == BPC - 1 and h == 1) else [H]
                    off = h * H
                    for w in widths:
                        nc.vector.copy_predicated(
                            out=xt[:, off : off + w],
                            mask=msh[:, off - h * H : off - h * H + w].bitcast(
                                mybir.dt.int32
                            ),
                            data=fills[b].to_broadcast([F, w]),
                        )
                        nc.scalar.dma_start(
                            out=ya[b][:, off : off + w],
                            in_=xt[:, off : off + w],
                        )
                        off += w

            # sample 0 emitted before everything else as ONE full-tile
            # DMA: each HWDGE completion on the ring adds ~2.1us of
            # serialized receipt latency, so a single DMA releases the
            # full-sample min earliest and moves every later sync-ring
            # sem up one slot
            emit_input(0, nchunks=1)

            # all samples' mask rows, loaded once up front on the scalar
            # ring, which is otherwise empty until the first store: their
            # completion sems don't queue behind x-input completions
            mtr_t = single.tile([2, BPC * T], BF16)
            nc.scalar.dma_start(out=mtr_t, in_=ta)
            mfl_t = single.tile([2, BPC * F], BF16)
            nc.scalar.dma_start(out=mfl_t, in_=fa)

            for b in range(BPC):
                if b == 0:
                    emit_reduce(0, 0, nchunks=1)
                elif b == 1:
                    emit_input(1)
                    emit_reduce(1, 0)
                    emit_fill(0)
                    emit_pred_store(0)
                    emit_reduce(1, 1)
                elif b < BPC - 1:
                    # steady state: one DMA + one full-tile reduce per
                    # sample (fewer per-op fixed costs on the DVE stream)
                    emit_input(b, nchunks=1)
                    emit_reduce(b, 0, nchunks=1)
                    emit_fill(b - 1)
                    emit_pred_store(b - 1)
                else:
                    # the LAST sample arrives as two halves: its h0
                    # reduce overlaps the h1 transfer, shortening the
                    # post-input tail chain (reduce+fill+preds) by ~1.6us
                    emit_input(b, nchunks=2)
                    emit_reduce(b, 0)
                    emit_fill(b - 1)
                    emit_pred_store(b - 1)
                    emit_reduce(b, 1)
            emit_fill(BPC - 1)
            emit_pred_store(BPC - 1)
    nc.compile()
    return nc


def _host_masks(f0, f_w, t0, t_w):
    nb = f0.shape[0]
    fidx = np.arange(F, dtype=np.int32)
    tidx = np.arange(T, dtype=np.int32)
    fm = (
        (fidx[None, None, :] >= f0[:, :, None])
        & (fidx[None, None, :] < (f0 + f_w)[:, :, None])
    ).any(axis=1)  # [B,F] bool
    tm = (
        (tidx[None, None, :] >= t0[:, :, None])
        & (tidx[None, None, :] < (t0 + t_w)[:, :, None])
    ).any(axis=1)  # [B,T] bool
    # row 0 of mtr = time masks concatenated, row 1 = ones;
    # row 0 of mfl = ones, row 1 = freq masks concatenated
    mtr = np.ones((2, nb * T), np.float32)
    mtr[0] = tm.reshape(-1)
    mfl = np.ones((2, nb * F), np.float32)
    mfl[1] = fm.reshape(-1)
    return (
        mtr.astype(ml_dtypes.bfloat16),
        mfl.astype(ml_dtypes.bfloat16),
    )


def _in_maps(x, f0, f_w, t0, t_w):
    x = np.ascontiguousarray(np.asarray(x, dtype=np.float32))
    mtr, mfl = _host_masks(
        np.asarray(f0), np.asarray(f_w), np.asarray(t0), np.asarray(t_w)
    )
    in_maps = []
    for c in range(N_CORES):
        in_maps.append(
            {
                "x_sh": np.ascontiguousarray(x[c * BPC : (c + 1) * BPC]),
                "mtr_sh": np.ascontiguousarray(mtr[:, c * BPC * T : (c + 1) * BPC * T]),
                "mfl_sh": np.ascontiguousarray(mfl[:, c * BPC * F : (c + 1) * BPC * F]),
            }
        )
    return in_maps


def kernel(x, f0, f_w, t0, t_w, **_):
    in_maps = _in_maps(x, f0, f_w, t0, t_w)

    if "nc" not in _cached:
        _cached["nc"] = _build_nc()
    nc = _cached["nc"]

    res = bass_utils.run_bass_kernel_spmd(
        nc, in_maps, core_ids=list(range(N_CORES))
    )
    out = np.concatenate([r["y_sh"] for r in res.results], axis=0)
    return out
